# revision 23
# baseline (speedup 1.0000x reference)
"""GCN (gather/segment-sum message passing) + mean-pool + MLP on 8 TRN2 cores.

Strategy (data-parallel over graphs, per the sharding hint):
 - nodes/graphs are sharded contiguously across 8 cores (batch is sorted);
   every edge is owned by the core owning its TARGET (col) node.
 - launch 1: each core computes y = rsqrt(deg) * (x @ W_gcn) for its node
   shard (host stages x pre-transposed so the PE contracts over in_dim).
 - host assembles the full y table, converts to bf16 and duplicates each
   row 4x so one gather descriptor moves 512B (descriptor-rate-bound SWDGE
   runs ~2x faster at 512B than 256B), + a zero row per bank.
 - launch 2: per core, per source-bank (int16 gather indices limit a table
   to 32k rows -> 4 banks), edges are organized into "prefix rounds": nodes
   sorted by per-bank in-degree, round r gathers the r-th in-edge source row
   of every node that has one. Each round's dma_gather output tile is then
   POSITION-ALIGNED with the accumulator (node rank i -> partition i%128,
   column i//128), so aggregation is plain DVE copies/adds - no scatter.
   Round 0 is full-width (C*128) and uses tensor_copy, so no acc memsets.
   Bank 0's partial acc is dumped to HBM zglob (plain strided write, rank
   order = final order); banks 1-3 merge via dma_scatter_add (CCE += in
   HBM, 512B descs via a trash half-row). zglob is read back contiguously,
   then z = relu(dinv*z + b), graph mean-pool via one-hot PSUM matmuls,
   and the 64->64->2 MLP + sigmoid, all on-chip. Output (64,2) per core.
"""

import os
import sys

sys.path.insert(0, "/opt/trn_rl_repo")

import numpy as np
import ml_dtypes

import concourse.bacc as bacc
import concourse.bass as bass
import concourse.mybir as mybir
import concourse.tile as tile
from concourse.bass_utils import run_bass_kernel_spmd

NC = 8          # cores
NB = 4          # source banks (int16 gather index limit)
CH = 1024       # gather chunk (slots per dma_gather; SWDGE ring cap)
NQ = 4          # SWDGE queues for dma_gather round-robin
SUP = 512       # idx super-tile columns (x16 idxs)
SCRATCH = 32768 # dynamic dma scratch (descriptor ring) bytes
P = 128
HID = 64
DUP = 4         # bf16 row duplication -> 512B descriptors
F32 = mybir.dt.float32
BF16 = mybir.dt.bfloat16
I16 = mybir.dt.int16

LAST_RUN_INFO = {}


def _split_multiwaits(nc, max_waits=1):
    """This walrus build rejects >1 semaphore wait per instruction; hoist
    extra waits onto same-engine NOPs placed immediately before."""
    import concourse.mybir as mb
    for f in nc.m.functions:
        for blk in f.blocks:
            insts = blk.instructions
            newlist = []
            changed = False
            for inst in insts:
                si = inst.sync_info
                waits = list(si.on_wait) if si is not None and si.on_wait else []
                if len(waits) > max_waits:
                    si.on_wait = waits[-max_waits:]
                    extra = waits[:-max_waits]
                    while extra:
                        nop = mb.InstNoOp(
                            name=f"I-mwsplit-{nc.next_id()}",
                            sync_info=mb.SyncInfo(on_wait=extra[:max_waits], on_update=[]),
                            engine=inst.engine,
                            bass_nofuse=True,
                        )
                        newlist.append(nop)
                        extra = extra[max_waits:]
                    changed = True
                newlist.append(inst)
            if changed:
                insts.clear()
                insts.extend(newlist)


_COMPILED = set()


def _run(nc, in_maps, trace=False):
    if id(nc) not in _COMPILED:
        nc.compile()
        _split_multiwaits(nc)
        _COMPILED.add(id(nc))
    kw = {}
    if trace:
        kw = dict(trace=True)
    try:
        return run_bass_kernel_spmd(nc, in_maps, list(range(NC)), **kw)
    except Exception:
        # transient device-unrecoverable (wedged core from an earlier run)
        import time as _time
        _time.sleep(10)
        return run_bass_kernel_spmd(nc, in_maps, list(range(NC)), **kw)


def _pjrt_runner(nc, in_maps):
    """Build the shard_map-jitted bass_exec callable ONCE with device-resident
    inputs; returns run_once() whose wall time is dispatch + device exec only
    (fresh donated zero-outputs are re-supplied per call; for benchmarking)."""
    import jax
    import numpy as _np
    from concourse import bass2jax as b2j

    b2j.install_neuronx_cc_hook()
    partition_name = nc.partition_id_tensor.name if nc.partition_id_tensor else None
    in_names, out_names, out_avals, zero_outs = [], [], [], []
    for alloc in nc.m.functions[0].allocations:
        if not isinstance(alloc, mybir.MemoryLocationSet):
            continue
        name = alloc.memorylocations[0].name
        if alloc.kind == "ExternalInput":
            if name != partition_name:
                in_names.append(name)
        elif alloc.kind == "ExternalOutput":
            shape = tuple(alloc.tensor_shape)
            dtype = mybir.dt.np(alloc.dtype)
            out_names.append(name)
            out_avals.append(jax.core.ShapedArray(shape, dtype))
            zero_outs.append(_np.zeros(shape, dtype))
    n_params, n_outs = len(in_names), len(out_avals)
    all_in = list(in_names) + out_names + ([partition_name] if partition_name else [])

    def _body(*args):
        operands = list(args)
        if partition_name is not None:
            operands.append(b2j.partition_id_tensor())
        outs = b2j._bass_exec_p.bind(
            *operands, out_avals=tuple(out_avals), in_names=tuple(all_in),
            out_names=tuple(out_names), lowering_input_output_aliases=(),
            sim_require_finite=True, sim_require_nnan=True, nc=nc)
        return tuple(outs)

    devices = jax.devices()[:NC]
    mesh = b2j.Mesh(_np.asarray(devices), ("core",))
    donate = tuple(range(n_params, n_params + n_outs))
    sharded = jax.jit(
        b2j.shard_map(_body, mesh=mesh,
                      in_specs=(b2j.PartitionSpec("core"),) * (n_params + n_outs),
                      out_specs=(b2j.PartitionSpec("core"),) * n_outs,
                      check_rep=False),
        donate_argnums=donate, keep_unused=True)
    concat_in = [
        jax.device_put(
            _np.concatenate([_np.asarray(m[name]) for m in in_maps], axis=0))
        for name in in_names
    ]
    for a in concat_in:
        a.block_until_ready()

    def run_once():
        zs = [_np.zeros((NC * z.shape[0], *z.shape[1:]), z.dtype) for z in zero_outs]
        outs = sharded(*concat_in, *zs)
        for o in outs:
            o.block_until_ready()
        return outs

    return run_once


# ---------------------------------------------------------------- launch 1


def _build_launch1(C):
    """y_tile = dinv * (x @ W);  x staged transposed [128(in), C*128(node)],
    bf16 in/out (the gather table is bf16 anyway)."""
    nc = bacc.Bacc("TRN2", target_bir_lowering=False, debug=False)
    xT = nc.declare_dram_parameter("xT", [P, C * P], BF16, isOutput=False)
    degn = nc.declare_dram_parameter("degn", [P, C], F32, isOutput=False)
    w = nc.declare_dram_parameter("w", [P, HID], BF16, isOutput=False)
    ysb = nc.declare_dram_parameter("ysb", [P, C * HID], BF16, isOutput=True)

    reps = int(os.environ.get("GCN_REPS", "1"))
    with tile.TileContext(nc) as tc:
        with (
            tc.tile_pool(name="sb", bufs=1) as sb,
            tc.tile_pool(name="sbx", bufs=3) as sbx,
            tc.tile_pool(name="ps", bufs=4, space="PSUM") as psp,
        ):
          for _rep in range(reps):
            w_t = sb.tile([P, HID], BF16)
            nc.scalar.dma_start(out=w_t[:], in_=w[:, :])
            deg_t = sb.tile([P, C], F32)
            nc.scalar.dma_start(out=deg_t[:], in_=degn[:, :])
            dinv = sb.tile([P, C], F32)
            nc.scalar.activation(dinv[:], deg_t[:], mybir.ActivationFunctionType.Sqrt)
            nc.vector.reciprocal(dinv[:], dinv[:])
            # load all of xT in a few big DMAs (100 tile-sized DMAs would pay
            # ~1us HWDGE fixed cost each); matmuls read slices.
            xt_all = sbx.tile([P, C * P], BF16, tag="xt_all")
            NXC = 4
            xb = (C + NXC - 1) // NXC
            for q in range(NXC):
                a, e = q * xb * P, min(C, (q + 1) * xb) * P
                if a < e:
                    nc.scalar.dma_start(out=xt_all[:, a:e], in_=xT[:, a:e])
            y_t = sb.tile([P, C, HID], BF16)
            for t in range(C):
                ps = psp.tile([P, HID], F32, space="PSUM")
                nc.tensor.matmul(out=ps[:], lhsT=xt_all[:, t * P:(t + 1) * P],
                                 rhs=w_t[:], start=True, stop=True)
                nc.vector.tensor_tensor(
                    out=y_t[:, t, :], in0=ps[:],
                    in1=dinv[:, t:t + 1].broadcast_to([P, HID]),
                    op=mybir.AluOpType.mult)
            nc.scalar.dma_start(out=ysb[:, :], in_=y_t[:].rearrange("p c h -> p (c h)"))
    return nc


# ---------------------------------------------------------------- launch 2


def _build_launch2(C, VB, bank_chunks, scat_off16, n_w16):
    """bank_chunks: per bank, list of (idx_off16, nidx, [(gcol, zcol, ncols, is_r0)..])
    scat_off16: per bank b>=1, idx column offset of its C*128 scatter-idx stream
    n_w16: total idx columns (int16 words / 16)."""
    nc = bacc.Bacc("TRN2", target_bir_lowering=False, debug=False,
                   num_swdge_queues=NQ, dynamic_dma_scratch_size=SCRATCH)
    ytab = nc.declare_dram_parameter("ytab", [NB * VB, HID * DUP], BF16,
                                     isOutput=False)
    idxs = nc.declare_dram_parameter("idxs", [P, n_w16], I16, isOutput=False)
    degz = nc.declare_dram_parameter("degz", [P, C], F32, isOutput=False)
    gl = nc.declare_dram_parameter("gl", [P, C], F32, isOutput=False)
    iota = nc.declare_dram_parameter("iota", [P, HID], F32, isOutput=False)
    brep = nc.declare_dram_parameter("brep", [P, HID], F32, isOutput=False)
    w1a = nc.declare_dram_parameter("w1a", [P, HID], F32, isOutput=False)
    w2a = nc.declare_dram_parameter("w2a", [P, 2], F32, isOutput=False)
    iden = nc.declare_dram_parameter("iden", [P, P], F32, isOutput=False)
    out = nc.declare_dram_parameter("out", [HID, 2], F32, isOutput=True)
    dbg = os.environ.get("GCN_DEBUG") == "1"
    if dbg:
        zdbg = nc.declare_dram_parameter("zdbg", [P, C * HID], F32, isOutput=True)
    NR = C * P
    zglob = nc.dram_tensor("zglob", [NR + P, 2 * HID], F32)

    reps = int(os.environ.get("GCN_REPS", "1"))
    with tile.TileContext(nc) as tc:
        with (
            tc.tile_pool(name="sb", bufs=1) as sb,
            tc.tile_pool(name="accp", bufs=1) as accp,
            tc.tile_pool(name="stage", bufs=int(os.environ.get("GCN_SBUFS", "4"))) as stage,
            tc.tile_pool(name="idxp", bufs=3) as idxp,
            tc.tile_pool(name="ohp", bufs=3) as ohp,
            tc.tile_pool(name="ps", bufs=1, space="PSUM") as psp,
            tc.tile_pool(name="ps2", bufs=1, space="PSUM") as psp2,
        ):
            # trash half-columns zeroed once (contents persist per tag/slot)
            # so the 512B scatter/dump rows carry finite (0) data there.
            z0 = accp.tile([P, C, 2 * HID], F32, tag="acc0")
            z1 = accp.tile([P, C, 2 * HID], F32, tag="acc1")
            nc.gpsimd.memset(z0[:, :, HID:2 * HID], 0.0)
            nc.gpsimd.memset(z1[:, :, HID:2 * HID], 0.0)

            sup_state = {"s0": -1, "tile": None}

            def get_idx(off16, w, back=False):
                if (sup_state["s0"] < 0 or off16 < sup_state["s0"]
                        or off16 + w > sup_state["s0"] + SUP):
                    # descending access patterns load a window ENDING here
                    base = max(0, off16 + w - SUP) if back else off16
                    w2 = min(SUP, n_w16 - base)
                    t = idxp.tile([P, SUP], I16, tag="idx")
                    nc.scalar.dma_start(out=t[:, :w2], in_=idxs[:, base:base + w2])
                    sup_state["s0"] = base
                    sup_state["tile"] = t
                o = off16 - sup_state["s0"]
                return sup_state["tile"][:, o:o + w]

            gq = [0]
            state = {"readback": None}

            def body():
              # (indented 2: repeated GCN_REPS times for benchmarking)
              sup_state["s0"] = -1
              acc0 = accp.tile([P, C, 2 * HID], F32, tag="acc0")
              acc1 = accp.tile([P, C, 2 * HID], F32, tag="acc1")
              accs = [acc0, acc1]
              sc_chunks = []       # (pos, ln, k) rank-range chunks
              pos = 0
              while pos < NR:
                  sc_chunks.append((pos, min(CH, NR - pos), len(sc_chunks)))
                  pos += sc_chunks[-1][1]
              dump_inst = None
              scat_insts = {}      # bank -> {chunk k: inst}
              for b in range(NB):
                  acc = accs[b % 2]
                  for (off16, nidx, pieces) in bank_chunks[b]:
                      it = get_idx(off16, nidx // 16)
                      st = stage.tile([P, CH // P, HID * DUP], BF16, tag="st")
                      gi = nc.gpsimd.dma_gather(
                          st[:, : nidx // P, :], ytab[b * VB:(b + 1) * VB, :],
                          it, nidx, nidx, HID * DUP, queue_num=gq[0] % NQ)
                      gq[0] += 1
                      for (gcol, zcol, ncols, is_r0) in pieces:
                          if is_r0:
                              nc.vector.tensor_copy(
                                  acc[:, zcol:zcol + ncols, 0:HID],
                                  st[:, gcol:gcol + ncols, 0:HID])
                          else:
                              nc.vector.tensor_tensor(
                                  out=acc[:, zcol:zcol + ncols, 0:HID],
                                  in0=acc[:, zcol:zcol + ncols, 0:HID],
                                  in1=st[:, gcol:gcol + ncols, 0:HID],
                                  op=mybir.AluOpType.add)
                  if b == 0:
                      # plain dump in rank order (= final node order)
                      dump_inst = nc.scalar.dma_start(
                          out=zglob[0:NR, :].rearrange("(c p) e -> p c e", p=P),
                          in_=acc[:])
                      if state["readback"] is not None:
                          for prb in state["readback"]:
                              tile.add_dep_helper(dump_inst.ins, prb.ins,
                                                  sync=True,
                                                  reason="dump overwrites zglob read last rep")
                  else:
                      # CCE scatter-add merge into zglob. Banks chunk the rank
                      # space identically, so RMW ordering is only needed
                      # chunk-wise across banks (chains run concurrently).
                      # Emit chunks in REVERSE: low ranks (high degree) get
                      # their last DVE add at the very end of the bank stream,
                      # so chunk 0 emitted first would head-of-line block the
                      # Pool queue; high-k chunks are ready much earlier.
                      sl = {}
                      for (pos, ln, k) in reversed(sc_chunks):
                          sit = get_idx(scat_off16[b] + pos // 16, ln // 16,
                                        back=True)
                          si = nc.gpsimd.dma_scatter_add(
                              zglob[:, :], acc[:, pos // P:(pos + ln) // P, :],
                              sit, ln, ln, 2 * HID, queue_num=gq[0] % NQ)
                          gq[0] += 1
                          deps = [dump_inst] if b == 1 else [scat_insts[b - 1][k]]
                          for d in deps:
                              tile.add_dep_helper(si.ins, d.ins, sync=True,
                                                  reason="zglob RMW ordering")
                          sl[k] = si
                      scat_insts[b] = sl
              # post-processing setup (independent of the merges)
              deg_t = sb.tile([P, C], F32)
              nc.scalar.dma_start(out=deg_t[:], in_=degz[:, :])
              dinv = sb.tile([P, C], F32)
              nc.scalar.activation(dinv[:], deg_t[:], mybir.ActivationFunctionType.Sqrt)
              nc.vector.reciprocal(dinv[:], dinv[:])
              brep_t = sb.tile([P, HID], F32)
              nc.scalar.dma_start(out=brep_t[:], in_=brep[:, :])
              gl_t = sb.tile([P, C], F32)
              nc.scalar.dma_start(out=gl_t[:], in_=gl[:, :])
              iota_t = sb.tile([P, HID], F32)
              nc.scalar.dma_start(out=iota_t[:], in_=iota[:, :])
              ones_t = sb.tile([P, 1], F32)
              nc.gpsimd.memset(ones_t[:], 1.0)
              ps_sum = psp.tile([HID, HID], F32, space="PSUM", tag="pssum")
              ps_cnt = psp.tile([HID, 1], F32, space="PSUM", tag="pscnt")
              # read back merged z (rank order) chunk-wise as each rank
              # range's scatter chain completes; post-process per chunk
              # (dinv scale + bias + relu + pooling matmuls). Chunks are
              # processed in REVERSE: high-k ranges (low degree) finish their
              # merge chains long before chunk 0 (touched by every round), so
              # descending order lets pooling overlap the remaining merges.
              # PSUM accumulation is order-independent; start/stop flags
              # follow emission order (first emitted col is C-1).
              zv = acc0[:, :, 0:HID]
              last_sc = scat_insts[NB - 1]
              rbs = []
              for (pos, ln, k) in reversed(sc_chunks):
                  c0, c1 = pos // P, (pos + ln) // P
                  rb = nc.scalar.dma_start(
                      out=acc0[:, c0:c1, :],
                      in_=zglob[pos:pos + ln, :].rearrange("(c p) e -> p c e", p=P))
                  tile.add_dep_helper(rb.ins, last_sc[k].ins, sync=True,
                                      reason="readback after chunk merge chain")
                  rbs.append(rb)
                  for c in range(c1 - 1, c0 - 1, -1):
                      nc.vector.tensor_tensor(
                          out=zv[:, c, :], in0=zv[:, c, :],
                          in1=dinv[:, c:c + 1].broadcast_to([P, HID]),
                          op=mybir.AluOpType.mult)
                      nc.vector.tensor_tensor(
                          out=zv[:, c, :], in0=zv[:, c, :], in1=brep_t[:],
                          op=mybir.AluOpType.add)
                  nc.scalar.activation(zv[:, c0:c1, :], zv[:, c0:c1, :],
                                       mybir.ActivationFunctionType.Relu)
                  for c in range(c1 - 1, c0 - 1, -1):
                      oh = ohp.tile([P, HID], F32, tag="oh")
                      nc.vector.tensor_tensor(
                          out=oh[:], in0=gl_t[:, c:c + 1].broadcast_to([P, HID]),
                          in1=iota_t[:], op=mybir.AluOpType.is_equal)
                      nc.tensor.matmul(out=ps_sum[:], lhsT=oh[:], rhs=zv[:, c, :],
                                       start=(c == C - 1), stop=(c == 0),
                                       skip_group_check=True)
                      nc.tensor.matmul(out=ps_cnt[:], lhsT=oh[:], rhs=ones_t[:],
                                       start=(c == C - 1), stop=(c == 0),
                                       skip_group_check=True)
              state["readback"] = rbs
              if dbg:
                  zc = sb.tile([P, C, HID], F32, tag="zdbg")
                  nc.vector.tensor_copy(zc[:], zv[:])
                  nc.scalar.dma_start(out=zdbg[:, :],
                                      in_=zc[:].rearrange("p c h -> p (c h)"))
              cnt = sb.tile([HID, 1], F32)
              nc.vector.tensor_scalar_max(cnt[:], ps_cnt[:], 1.0)
              nc.vector.reciprocal(cnt[:], cnt[:])
              g_sb = sb.tile([HID, HID], F32)
              nc.vector.tensor_tensor(out=g_sb[:], in0=ps_sum[:],
                                      in1=cnt[:].broadcast_to([HID, HID]),
                                      op=mybir.AluOpType.mult)
              # MLP with homogeneous-coordinate bias
              iden_t = sb.tile([P, P], F32)
              nc.scalar.dma_start(out=iden_t[:], in_=iden[:, :])
              w1_t = sb.tile([P, HID], F32)
              nc.scalar.dma_start(out=w1_t[:], in_=w1a[:, :])
              w2_t = sb.tile([P, 2], F32)
              nc.scalar.dma_start(out=w2_t[:], in_=w2a[:, :])
              gT = psp2.tile([HID, HID], F32, space="PSUM", tag="tr")
              nc.tensor.transpose(out=gT[:], in_=g_sb[:], identity=iden_t[:HID, :HID])
              a1 = sb.tile([P, HID], F32)
              nc.gpsimd.memset(a1[HID:HID + 1, :], 1.0)
              nc.vector.tensor_copy(a1[:HID, :], gT[:])
              h_ps = psp2.tile([HID, HID], F32, space="PSUM", tag="mm")
              nc.tensor.matmul(out=h_ps[:], lhsT=a1[0:HID + 1, :], rhs=w1_t[0:HID + 1, :],
                               start=True, stop=True)
              h_sb = sb.tile([HID, HID], F32)
              nc.scalar.activation(h_sb[:], h_ps[:], mybir.ActivationFunctionType.Relu)
              hT = psp2.tile([HID, HID], F32, space="PSUM", tag="tr2")
              nc.tensor.transpose(out=hT[:], in_=h_sb[:], identity=iden_t[:HID, :HID])
              a2 = sb.tile([P, HID], F32)
              nc.gpsimd.memset(a2[HID:HID + 1, :], 1.0)
              nc.vector.tensor_copy(a2[:HID, :], hT[:])
              o_ps = psp2.tile([HID, 2], F32, space="PSUM", tag="mm2")
              nc.tensor.matmul(out=o_ps[:], lhsT=a2[0:HID + 1, :], rhs=w2_t[0:HID + 1, :],
                               start=True, stop=True)
              o_sb = sb.tile([HID, 2], F32)
              nc.scalar.activation(o_sb[:], o_ps[:], mybir.ActivationFunctionType.Sigmoid)
              nc.scalar.dma_start(out=out[:, :], in_=o_sb[:])

            for _rep in range(reps):
                body()
    return nc


# ---------------------------------------------------------------- host glue


def _wrap16(vals):
    """int16 stream -> [128, ceil(n/16)] ucode layout (16-wrapped, 8x repl)."""
    n = len(vals)
    w = (n + 15) // 16
    a = np.full(w * 16, -1, np.int16)
    a[:n] = vals
    blk = a.reshape(w, 16).T
    return np.tile(blk, (8, 1))


def kernel(x, edge_index, batch, W_gcn, b_gcn, W1, b1, W2, b2):
    x = np.ascontiguousarray(np.asarray(x, dtype=np.float32))
    ei = np.asarray(edge_index).astype(np.int64)
    batch_np = np.asarray(batch).astype(np.int64)
    W_gcn = np.asarray(W_gcn, np.float32); b_gcn = np.asarray(b_gcn, np.float32)
    W1 = np.asarray(W1, np.float32); b1 = np.asarray(b1, np.float32)
    W2 = np.asarray(W2, np.float32); b2 = np.asarray(b2, np.float32)

    N = x.shape[0]
    G = 512
    BS = (N + NB - 1) // NB          # nodes per source bank
    VB = BS + 1                      # +1 zero row per bank
    row = ei[0].astype(np.int64)
    col = ei[1].astype(np.int64)
    # self loops appended
    sl = np.arange(N, dtype=np.int64)
    row2 = np.concatenate([row, sl])
    col2 = np.concatenate([col, sl])
    deg = np.bincount(col2, minlength=N).astype(np.float32)  # >=1 always

    gpc = G // NC
    gb = np.searchsorted(batch_np, np.arange(0, G + 1, gpc))
    Ncs = np.diff(gb)
    C = int((Ncs.max() + P - 1) // P)
    NR = C * P

    # ---------------- launch 1: y shards
    in1 = []
    W_bf = W_gcn.astype(ml_dtypes.bfloat16)
    for c in range(NC):
        lo, hi = int(gb[c]), int(gb[c + 1])
        n = hi - lo
        xT = np.zeros((P, C * P), ml_dtypes.bfloat16)
        xT[:, :n] = x[lo:hi].T
        dg = np.ones((P, C), np.float32)
        dgf = dg.reshape(-1, order="F")      # (p,t) -> t*128+p
        dgf[:n] = deg[lo:hi]
        dg = dgf.reshape(C, P).T.copy()
        in1.append({"xT": xT, "degn": dg, "w": W_bf})
    nc1 = _build_launch1(C)
    trace = os.environ.get("GCN_TRACE") == "1"
    r1 = _run(nc1, in1, trace=trace)
    LAST_RUN_INFO["exec1_ns"] = r1.exec_time_ns
    y_bf = np.empty((N, HID), ml_dtypes.bfloat16)
    for c in range(NC):
        lo, hi = int(gb[c]), int(gb[c + 1])
        ys = r1.results[c]["ysb"].reshape(P, C, HID).transpose(1, 0, 2).reshape(-1, HID)
        y_bf[lo:hi] = ys[: hi - lo]
    # bf16 table with DUP-duplicated rows (512B gather descriptors)
    ytab = np.zeros((NB * VB, HID * DUP), ml_dtypes.bfloat16)
    for b in range(NB):
        nlo, nhi = b * BS, min((b + 1) * BS, N)
        ytab[b * VB: b * VB + (nhi - nlo)] = np.tile(y_bf[nlo:nhi], (1, DUP))

    # ---------------- per-core schedules (common across cores)
    core_data = []
    for c in range(NC):
        lo, hi = int(gb[c]), int(gb[c + 1])
        m = (col2 >= lo) & (col2 < hi)
        r_c = row2[m]
        cl = (col2[m] - lo).astype(np.int64)
        bank = np.minimum(r_c // BS, NB - 1)
        core_data.append((lo, hi, r_c, cl, bank))

    # common round schedule per bank: N_br = max over cores of roundup128(n_br)
    nbr_all = []          # [NB][core] -> array of n_br
    for b in range(NB):
        per_core = []
        for c in range(NC):
            lo, hi, r_c, cl, bank = core_data[c]
            nloc = hi - lo
            degb = np.bincount(cl[bank == b], minlength=nloc)
            if degb.max() == 0:
                per_core.append(np.zeros(0, np.int64))
                continue
            h = np.bincount(degb)            # h[d] = #nodes with degb == d
            # n_br = #{deg_b > r} for r = 0..max-1
            nbr = (nloc - np.cumsum(h))[:len(h) - 1]
            per_core.append(np.asarray(nbr, np.int64))
        nbr_all.append(per_core)
    bank_rounds = []      # [NB] -> padded common N_br (cols of 128)
    for b in range(NB):
        R = max((len(a) for a in nbr_all[b]), default=0)
        Nbr = np.zeros(R, np.int64)
        for a in nbr_all[b]:
            aa = np.zeros(R, np.int64)
            aa[:len(a)] = a
            Nbr = np.maximum(Nbr, ((aa + P - 1) // P) * P)
        if len(Nbr) == 0:
            Nbr = np.zeros(1, np.int64)
        Nbr[0] = NR       # round 0 full width: tensor_copy covers every rank
        bank_rounds.append(Nbr)

    # chunk schedule (common): per bank, chunks of <=CH slots + round pieces
    bank_chunks = []
    bank_off16 = []       # idx tensor column offset for each bank stream
    off16 = 0
    for b in range(NB):
        Nbr = bank_rounds[b]
        S = int(Nbr.sum())
        starts = np.concatenate([[0], np.cumsum(Nbr)])
        chunks = []
        pos = 0
        while pos < S:
            ln = min(CH, S - pos)
            pieces = []
            for r in range(len(Nbr)):
                a = max(pos, starts[r]); e = min(pos + ln, starts[r + 1])
                if a < e:
                    pieces.append((int((a - pos) // P), int((a - starts[r]) // P),
                                   int((e - a) // P), bool(r == 0)))
            chunks.append((off16 + pos // 16, int(ln), pieces))
            pos += ln
        bank_chunks.append(chunks)
        bank_off16.append(off16)
        off16 += S // 16
    # scatter idx streams (banks 1..NB-1): NR idxs per bank
    scat_off16 = [0] * NB
    for b in range(1, NB):
        scat_off16[b] = off16
        off16 += NR // 16
    n_w16 = off16

    # ---------------- per-core idx streams
    in2 = []
    iota64 = np.tile(np.arange(HID, dtype=np.float32), (P, 1))
    brep = np.tile(b_gcn[None, :], (P, 1)).astype(np.float32)
    w1a = np.zeros((P, HID), np.float32); w1a[:HID] = W1; w1a[HID] = b1
    w2a = np.zeros((P, 2), np.float32); w2a[:HID] = W2; w2a[HID] = b2
    iden = np.eye(P, dtype=np.float32)
    for c in range(NC):
        lo, hi, r_c, cl, bank = core_data[c]
        nloc = hi - lo
        idxbuf = np.full(n_w16 * 16, -1, np.int16)
        ranks = []            # per bank: node -> bank rank
        for b in range(NB):
            Nbr = bank_rounds[b]
            S = int(Nbr.sum())
            starts = np.concatenate([[0], np.cumsum(Nbr)])
            stream = np.full(S, BS, np.int16)          # dummy -> zero row
            mb = bank == b
            rb, clb = r_c[mb], cl[mb]
            degb = np.bincount(clb, minlength=nloc)
            order = np.argsort(-degb, kind="stable")   # bank-rank -> node
            rank = np.empty(nloc, np.int64)
            rank[order] = np.arange(nloc)
            ranks.append(rank)
            rk = rank[clb]
            o = np.lexsort((np.arange(len(rk)), rk))
            rk_s, src_s = rk[o], (rb[o] - b * BS)
            grp_start = np.searchsorted(rk_s, rk_s)    # first occurrence index
            j = np.arange(len(rk_s)) - grp_start
            stream[starts[j] + rk_s] = src_s.astype(np.int16)
            idxbuf[bank_off16[b] * 16: bank_off16[b] * 16 + S] = stream
        # scatter idx: bank-b rank r -> bank-0 rank of same node (dead -> NR)
        r0 = ranks[0]
        for b in range(1, NB):
            rank_b = ranks[b]
            order_b = np.empty(nloc, np.int64)
            order_b[rank_b] = np.arange(nloc)          # bank rank -> node
            mstream = np.full(NR, NR, np.int16)        # dead row
            mstream[:nloc] = r0[order_b].astype(np.int16)
            idxbuf[scat_off16[b] * 16: scat_off16[b] * 16 + NR] = mstream
        idxw = _wrap16(idxbuf)                          # [128, n_w16]
        # aux streams in bank-0 rank order
        order0 = np.empty(nloc, np.int64)
        order0[r0] = np.arange(nloc)                    # bank0 rank -> node
        LAST_RUN_INFO.setdefault("order0s", {})[c] = order0
        dgz = np.ones(NR, np.float32)
        dgz[:nloc] = deg[lo:hi][order0]
        glv = np.full(NR, float(HID), np.float32)
        glv[:nloc] = (batch_np[lo:hi][order0] - c * gpc).astype(np.float32)
        in2.append({
            "ytab": ytab, "idxs": idxw,
            "degz": dgz.reshape(C, P).T.copy(),
            "gl": glv.reshape(C, P).T.copy(),
            "iota": iota64, "brep": brep, "w1a": w1a, "w2a": w2a, "iden": iden,
        })

    LAST_RUN_INFO["launch2_args"] = (C, VB, bank_chunks, scat_off16, n_w16)
    LAST_RUN_INFO["in2"] = in2
    LAST_RUN_INFO["in1"] = in1
    LAST_RUN_INFO["C"] = C
    nc2 = _build_launch2(C, VB, bank_chunks, scat_off16, n_w16)
    r2 = _run(nc2, in2, trace=trace)
    LAST_RUN_INFO["exec2_ns"] = r2.exec_time_ns
    if os.environ.get("GCN_DEBUG") == "1":
        LAST_RUN_INFO["zdbg"] = [r2.results[c]["zdbg"].reshape(P, C, HID) for c in range(NC)]
        LAST_RUN_INFO["gb"] = gb
    out = np.concatenate([r2.results[c]["out"] for c in range(NC)], axis=0)
    return out[:G].astype(np.float32)
